# revision 12
# baseline (speedup 1.0000x reference)
"""3-layer GAT (PyG GATConv semantics) on 8 Trainium2 NeuronCores.

Strategy (dst-sharded, gather-based):
- Nodes are assigned to 160 blocks of <=128 dst nodes, degree-balanced; 20 blocks per core.
- Per layer: each core computes its shard's dense projection h_aug = hprev @ [W | W@a_src | W@a_dst]
  (f32r matmuls), writes an fp16 row table [slots, 264], AllGathers the table.
- Edge phase per block: indirect-DMA gather of h_aug rows by edge src (fp16, 528B rows),
  indirect gather of alpha_dst rows by edge dst (f32, 16B rows), per-edge
  e = leaky(alpha_src + alpha_dst), ex = exp(e) (f32 math), messages m = ex * h (fp16),
  aggregation + softmax denominators via one PE matmul per 128-edge tile:
  lhsT = S (0/1 edge->dstslot matrix built by iota-compare), rhs = [m | ex] -> PSUM [128, 260].
- Softmax applied after aggregation: out = psum[:, :256] / denom (per head), + bias, ELU.
- Layer 3 (heads=1, C=1) same scheme with scalar tables.

The walrus in this toolchain accepts only ONE sync wait per instruction; BassOneWait
splits Tile-generated multi-waits into single-wait EventSemaphore ops at serialization.
"""
import numpy as np
from contextlib import ExitStack
import heapq

import orjson
import concourse.bass as bass
import concourse.tile as tile
from concourse import mybir
from concourse.bass_utils import run_bass_kernel_spmd

# problem constants (fixed by the harness's setup_inputs)
N_NODES = 20000
N_EDGES = 320000
IN_DIM = 128
HID = 64
HEADS = 4
HC = HEADS * HID          # 256
AUG = HC + 2 * HEADS      # 264 = h | alpha_src | alpha_dst
NEG = 0.2
NCORES = 8
P = 128
NBLK = 20                 # dst blocks per core
SLOTS = NBLK * P          # 2560 slots per core
TOT_SLOTS = SLOTS * NCORES

F32 = mybir.dt.float32
F32R = mybir.dt.float32r
F16 = mybir.dt.float16
I32 = mybir.dt.int32


def _split_multiwaits(bir: bytes) -> bytes:
    """Walrus here allows only 1 sync wait per instruction -> hoist extras onto
    same-engine EventSemaphore waits. Additionally, qPoolDynamic (SWDGE) DMA
    descriptors do not reliably honor embedded waits under this runtime ->
    hoist ALL their waits onto the issuing Pool engine."""
    j = orjson.loads(bir)
    ctr = 0
    for fn in j["functions"]:
        for blk in fn["blocks"]:
            out_l = []
            for ins in blk["instructions"]:
                si = ins.get("sync_info")
                ow = (si or {}).get("on_wait") or []
                hoist_all = ins.get("opcode") == "DMACopy" and ins.get("queue") == "qPoolDynamic"
                keep = 0 if hoist_all else 1
                if len(ow) > keep:
                    for w in ow[:len(ow) - keep]:
                        ctr += 1
                        out_l.append({
                            "engine": ins["engine"], "ins": [], "outs": [],
                            "name": f"mwsplit-{ctr}", "opcode": "EventSemaphore",
                            "sync_info": {"on_update": [], "on_wait": [w]},
                        })
                    si["on_wait"] = ow[len(ow) - keep:]
                out_l.append(ins)
            blk["instructions"] = out_l
    return orjson.dumps(j)


class BassOneWait(bass.Bass):
    def to_json_bytes(self):
        return _split_multiwaits(super().to_json_bytes())


# ---------------------------------------------------------------- host prep

def _preprocess(edge_index):
    """Assign nodes to degree-balanced blocks; build per-core edge tile arrays."""
    src = np.asarray(edge_index[0], dtype=np.int64)
    dst = np.asarray(edge_index[1], dtype=np.int64)
    loops = np.arange(N_NODES, dtype=np.int64)
    src = np.concatenate([src, loops])
    dst = np.concatenate([dst, loops])
    deg = np.bincount(dst, minlength=N_NODES).astype(np.int64)

    NB_TOT = NCORES * NBLK
    # greedy LPT: highest degree first onto least-loaded block with space
    order = np.argsort(-deg, kind="stable")
    blk_of = np.empty(N_NODES, np.int32)
    slot_of = np.empty(N_NODES, np.int32)
    heap = [(0, 0, b) for b in range(NB_TOT)]
    heapq.heapify(heap)
    cnt = np.zeros(NB_TOT, np.int32)
    load = np.zeros(NB_TOT, np.int64)
    for n in order:
        while True:
            l, _, b = heapq.heappop(heap)
            if cnt[b] < P:
                break
        blk_of[n] = b
        slot_of[n] = cnt[b]
        cnt[b] += 1
        load[b] += deg[n]
        if cnt[b] < P:
            heapq.heappush(heap, (load[b], cnt[b], b))

    T = int(np.ceil(load.max() / P))  # edge tiles per block (same for all)
    gslot = blk_of.astype(np.int64) * P + slot_of        # global table row of node
    node_of_slot = np.full(NB_TOT * P, -1, np.int64)
    node_of_slot[gslot] = np.arange(N_NODES)

    # bucket edges by dst block
    eb = blk_of[dst]
    order_e = np.argsort(eb, kind="stable")
    src_s = src[order_e]
    dst_s = dst[order_e]
    eb_s = eb[order_e]
    starts = np.searchsorted(eb_s, np.arange(NB_TOT + 1))

    NT = NBLK * T
    srcg = np.zeros((NCORES, P, NT), np.int32)       # global table row of edge src
    dstl = np.zeros((NCORES, P, NT), np.int32)       # core-local slot of edge dst
    dblk = np.full((NCORES, P, NT), -1.0, np.float16)  # block-local dst slot (-1 pad)
    for b in range(NB_TOT):
        c, lb = divmod(b, NBLK)
        e0, e1 = starts[b], starts[b + 1]
        k = e1 - e0
        col = np.zeros(T * P, np.int64)
        col[:k] = gslot[src_s[e0:e1]]
        srcg[c, :, lb * T:(lb + 1) * T] = col.reshape(T, P).T
        col_d = np.zeros(T * P, np.int64)
        col_d[:k] = lb * P + slot_of[dst_s[e0:e1]]
        dstl[c, :, lb * T:(lb + 1) * T] = col_d.reshape(T, P).T
        col_b = np.full(T * P, -1.0, np.float32)
        col_b[:k] = slot_of[dst_s[e0:e1]]
        dblk[c, :, lb * T:(lb + 1) * T] = col_b.reshape(T, P).T.astype(np.float16)

    return T, gslot, node_of_slot, srcg, dstl, dblk


def _aug_weights(W, a_src, a_dst, heads, hid):
    """[W | ws | wd] with ws[:,h] = W[:, h*hid:(h+1)*hid] @ a_src[h]."""
    cin = W.shape[0]
    ws = np.zeros((cin, heads), np.float32)
    wd = np.zeros((cin, heads), np.float32)
    for h in range(heads):
        blk = W[:, h * hid:(h + 1) * hid]
        ws[:, h] = blk @ a_src[h]
        wd[:, h] = blk @ a_dst[h]
    return np.concatenate([W, ws, wd], axis=1).astype(np.float32)


# ---------------------------------------------------------------- device kernel

def _build(T):
    NT = NBLK * T
    nc = BassOneWait()
    dp = nc.declare_dram_parameter
    x_in = dp("x_in", [SLOTS, IN_DIM], F32, isOutput=False)
    srcg_in = dp("srcg_in", [P, NT], I32, isOutput=False)
    dstl_in = dp("dstl_in", [P, NT], I32, isOutput=False)
    dblk_in = dp("dblk_in", [P, NT], F16, isOutput=False)
    wa1_in = dp("wa1_in", [IN_DIM, AUG], F32, isOutput=False)
    wa2_in = dp("wa2_in", [HC, AUG], F32, isOutput=False)
    w3_in = dp("w3_in", [1, HC], F32, isOutput=False)
    c3_in = dp("c3_in", [1, 4], F32, isOutput=False)   # a_src3, a_dst3, b3, 0
    b1_in = dp("b1_in", [1, HC], F32, isOutput=False)
    b2_in = dp("b2_in", [1, HC], F32, isOutput=False)
    iota_in = dp("iota_in", [1, P], F16, isOutput=False)
    ident_in = dp("ident_in", [P, P], F32, isOutput=False)
    out_p = dp("out_p", [P, NBLK], F32, isOutput=True)

    # internal DRAM
    tab_sh = [nc.dram_tensor(f"tab_sh{l}", [SLOTS, AUG], F16) for l in (1, 2)]
    tab_full = [nc.dram_tensor(f"tab_full{l}", [TOT_SLOTS, AUG], F16) for l in (1, 2)]
    adl_dram = [nc.dram_tensor(f"adl{l}", [SLOTS, HEADS], F32) for l in (1, 2)]
    h3_sh = nc.dram_tensor("h3_sh", [SLOTS, 1], F32)
    tab3 = nc.dram_tensor("tab3", [TOT_SLOTS, 1], F32)

    groups = [list(range(NCORES))]

    with tile.TileContext(nc) as tc, ExitStack() as ctx:
        consts = ctx.enter_context(tc.tile_pool(name="consts", bufs=1))
        meta = ctx.enter_context(tc.tile_pool(name="meta", bufs=1))
        state = ctx.enter_context(tc.tile_pool(name="state", bufs=1))
        work = ctx.enter_context(tc.tile_pool(name="work", bufs=2))
        gpool = ctx.enter_context(tc.tile_pool(name="gpool", bufs=3))
        small = ctx.enter_context(tc.tile_pool(name="small", bufs=4))
        psd = ctx.enter_context(tc.tile_pool(name="psd", bufs=2, space="PSUM"))
        pse = ctx.enter_context(tc.tile_pool(name="pse", bufs=2, space="PSUM"))
        pst = ctx.enter_context(tc.tile_pool(name="pst", bufs=2, space="PSUM"))

        # ---- constants / metadata loads
        ident = consts.tile([P, P], F32)
        nc.sync.dma_start(out=ident, in_=ident_in[:])
        wa1 = consts.tile([P, AUG], F32)
        nc.sync.dma_start(out=wa1, in_=wa1_in[:])
        wa2 = consts.tile([P, 2, AUG], F32)
        nc.sync.dma_start(out=wa2, in_=wa2_in.rearrange("(j p) a -> p j a", p=P))
        def rep_load(name, src, n, dt):
            t = consts.tile([P, n], dt, tag=name)
            bc = bass.AP(tensor=src.tensor, offset=0, ap=[[0, P], [1, n]])
            nc.sync.dma_start(out=t, in_=bc)
            return t
        w3r = rep_load("w3r", w3_in[:], HC, F32)
        c3 = rep_load("c3", c3_in[:], 4, F32)
        b1r = rep_load("b1r", b1_in[:], HC, F32)
        b2r = rep_load("b2r", b2_in[:], HC, F32)
        iot = rep_load("iot", iota_in[:], P, F16)

        srcg = meta.tile([P, NT], I32)
        nc.sync.dma_start(out=srcg, in_=srcg_in[:])
        dstl = meta.tile([P, NT], I32)
        nc.sync.dma_start(out=dstl, in_=dstl_in[:])
        dblk = meta.tile([P, NT], F16)
        nc.sync.dma_start(out=dblk, in_=dblk_in[:])

        xin = state.tile([P, NBLK, IN_DIM], F32)
        nc.sync.dma_start(out=xin, in_=x_in.rearrange("(b p) d -> p b d", p=P))

        hprev = state.tile([P, NBLK, HC], F32)   # layer-1 output
        hprev2 = state.tile([P, NBLK, HC], F32)  # layer-2 output
        hT = state.tile([P, 2 * NBLK, P], F32)   # transposed dense input

        def bcast_row(t, shape):
            # t is [P, n] partition-replicated; broadcast middle dims (stride 0)
            ap = [list(t.ap[0])]
            for s in shape[1:-1]:
                ap.append([0, s])
            ap.append([t.ap[-1][0], shape[-1]])
            return bass.AP(tensor=t.tensor, offset=t.offset, ap=ap)

        def dense_layer(lidx, cin_tiles):
            """h_aug per block -> tab_sh[lidx], adl_dram[lidx]."""
            adl_sb = state.tile([P, NBLK, HEADS], F32, tag=f"adl_sb{lidx}")
            for b in range(NBLK):
                ps = psd.tile([P, AUG], F32, tag="dense")
                for j in range(cin_tiles):
                    lhsT = hT[:, cin_tiles * b + j, :]
                    rhs = wa1[:, :] if lidx == 0 else wa2[:, j, :]
                    nc.tensor.matmul(ps, lhsT, rhs,
                                     start=(j == 0), stop=(j == cin_tiles - 1))
                tabt = small.tile([P, AUG], F16, tag="tabt")
                nc.vector.tensor_copy(out=tabt, in_=ps)
                nc.sync.dma_start(
                    out=tab_sh[lidx].rearrange("(b p) a -> p b a", p=P)[:, b, :],
                    in_=tabt)
                nc.vector.tensor_copy(out=adl_sb[:, b, :], in_=ps[:, HC + HEADS:AUG])
            nc.sync.dma_start(
                out=adl_dram[lidx].rearrange("(b p) h -> p b h", p=P), in_=adl_sb)

        def transpose_into(src_view, dst_col):
            """PE-transpose [128,128] src_view into hT[:, dst_col, :]."""
            tp = pst.tile([P, P], F32, tag="tr")
            nc.tensor.transpose(out=tp, in_=src_view, identity=ident)
            nc.vector.tensor_copy(out=hT[:, dst_col, :], in_=tp)

        def edge_layer(lidx, hout, brow):
            """Gather + attention + aggregate for layer lidx (0 or 1)."""
            for b in range(NBLK):
                sl = slice(b * T, (b + 1) * T)
                hg = gpool.tile([P, T, AUG], F16, tag="hg")
                adx = gpool.tile([P, T, HEADS], F32, tag="adx")
                for t in range(T):
                    gt = b * T + t
                    nc.gpsimd.indirect_dma_start(
                        out=hg[:, t, :], out_offset=None, in_=tab_full[lidx][:],
                        in_offset=bass.IndirectOffsetOnAxis(ap=srcg[:, gt:gt+1], axis=0))
                    nc.gpsimd.indirect_dma_start(
                        out=adx[:, t, :], out_offset=None, in_=adl_dram[lidx][:],
                        in_offset=bass.IndirectOffsetOnAxis(ap=dstl[:, gt:gt+1], axis=0))

                asum = small.tile([P, T, HEADS], F32, tag="asum")
                nc.vector.tensor_copy(out=asum, in_=hg[:, :, HC:HC + HEADS])
                nc.vector.tensor_tensor(out=asum, in0=asum, in1=adx,
                                        op=mybir.AluOpType.add)
                lk = small.tile([P, T, HEADS], F32, tag="lk")
                nc.vector.tensor_scalar_mul(lk, asum, NEG)
                nc.vector.tensor_tensor(out=lk, in0=lk, in1=asum,
                                        op=mybir.AluOpType.max)
                exf = small.tile([P, T, HEADS], F16, tag="exf")
                nc.scalar.activation(out=exf, in_=lk,
                                     func=mybir.ActivationFunctionType.Exp)

                m = work.tile([P, T, HC + HEADS], F16, tag="m")
                ex_b = bass.AP(tensor=exf.tensor, offset=exf.offset,
                               ap=[exf.ap[0], exf.ap[1], exf.ap[2], [0, HID]])
                nc.vector.tensor_tensor(
                    out=m[:, :, 0:HC].rearrange("p t (h c) -> p t h c", h=HEADS),
                    in0=hg[:, :, 0:HC].rearrange("p t (h c) -> p t h c", h=HEADS),
                    in1=ex_b, op=mybir.AluOpType.mult)
                nc.vector.tensor_copy(out=m[:, :, HC:HC + HEADS], in_=exf)

                S = work.tile([P, T, P], F16, tag="S")
                db_b = bass.AP(tensor=dblk.tensor, offset=dblk[:, sl].offset,
                               ap=[dblk.ap[0], [dblk.ap[1][0], T], [0, P]])
                nc.vector.tensor_tensor(out=S, in0=db_b,
                                        in1=bcast_row(iot, [P, T, P]),
                                        op=mybir.AluOpType.is_equal)

                ps = pse.tile([P, HC + HEADS], F32, tag="agg")
                for t in range(T):
                    nc.tensor.matmul(ps, S[:, t, :], m[:, t, :],
                                     start=(t == 0), stop=(t == T - 1))

                den = small.tile([P, HEADS], F32, tag="den")
                nc.vector.tensor_scalar_max(den, ps[:, HC:HC + HEADS], 1e-30)
                rec = small.tile([P, HEADS], F32, tag="rec")
                nc.vector.reciprocal(out=rec, in_=den)
                rec_b = bass.AP(tensor=rec.tensor, offset=rec.offset,
                                ap=[rec.ap[0], rec.ap[1], [0, HID]])
                hn = small.tile([P, HC], F32, tag="hn")
                nc.vector.tensor_tensor(
                    out=hn.rearrange("p (h c) -> p h c", h=HEADS),
                    in0=ps[:, 0:HC].rearrange("p (h c) -> p h c", h=HEADS),
                    in1=rec_b, op=mybir.AluOpType.mult)
                # bias + ELU
                nc.vector.tensor_tensor(out=hn, in0=hn, in1=brow,
                                        op=mybir.AluOpType.add)
                emin = small.tile([P, HC], F32, tag="emin")
                nc.vector.tensor_scalar_min(emin, hn, 0.0)
                eex = small.tile([P, HC], F32, tag="eex")
                nc.scalar.activation(out=eex, in_=emin,
                                     func=mybir.ActivationFunctionType.Exp)
                nc.vector.tensor_scalar_max(hn, hn, 0.0)
                nc.vector.tensor_tensor(out=hn, in0=hn, in1=eex,
                                        op=mybir.AluOpType.add)
                nc.vector.tensor_scalar_add(hout[:, b, :], hn, -1.0)

        # ================= layer 1
        for b in range(NBLK):
            transpose_into(xin[:, b, :], b)
        dense_layer(0, 1)
        nc.gpsimd.collective_compute(
            "AllGather", mybir.AluOpType.bypass, replica_groups=groups,
            ins=[tab_sh[0][:]], outs=[tab_full[0][:]])
        edge_layer(0, hprev, b1r)

        # ================= layer 2
        for b in range(NBLK):
            transpose_into(hprev[:, b, 0:P], 2 * b)
            transpose_into(hprev[:, b, P:HC], 2 * b + 1)
        dense_layer(1, 2)
        nc.gpsimd.collective_compute(
            "AllGather", mybir.AluOpType.bypass, replica_groups=groups,
            ins=[tab_sh[1][:]], outs=[tab_full[1][:]])
        edge_layer(1, hprev2, b2r)

        # ================= layer 3 dense: h3 = hprev2 @ W3 + b3
        h3sb = state.tile([P, NBLK, 1], F32)
        for b in range(NBLK):
            tmp = small.tile([P, HC], F32, tag="l3tmp")
            nc.vector.tensor_tensor(out=tmp, in0=hprev2[:, b, :],
                                    in1=w3r,
                                    op=mybir.AluOpType.mult)
            nc.vector.tensor_reduce(out=h3sb[:, b, :], in_=tmp,
                                    axis=mybir.AxisListType.X,
                                    op=mybir.AluOpType.add)
        b3_b = bass.AP(tensor=c3.tensor, offset=c3[:, 2:3].offset,
                       ap=[list(c3.ap[0]), [0, NBLK], [0, 1]])
        nc.vector.tensor_tensor(out=h3sb, in0=h3sb, in1=b3_b,
                                op=mybir.AluOpType.add)
        nc.sync.dma_start(out=h3_sh.rearrange("(b p) o -> p b o", p=P), in_=h3sb)
        nc.gpsimd.collective_compute(
            "AllGather", mybir.AluOpType.bypass, replica_groups=groups,
            ins=[h3_sh[:]], outs=[tab3[:]])

        # ================= layer 3 edge phase
        outsb = state.tile([P, NBLK], F32)
        a3s_b = lambda sh: bass.AP(tensor=c3.tensor, offset=c3[:, 0:1].offset,
                                   ap=[list(c3.ap[0]), [0, sh[1]], [0, 1]])
        a3d_b = lambda sh: bass.AP(tensor=c3.tensor, offset=c3[:, 1:2].offset,
                                   ap=[list(c3.ap[0]), [0, sh[1]], [0, 1]])
        for b in range(NBLK):
            sl = slice(b * T, (b + 1) * T)
            g3 = gpool.tile([P, T, 1], F32, tag="g3")
            d3 = gpool.tile([P, T, 1], F32, tag="d3")
            for t in range(T):
                gt = b * T + t
                nc.gpsimd.indirect_dma_start(
                    out=g3[:, t, :], out_offset=None, in_=tab3[:],
                    in_offset=bass.IndirectOffsetOnAxis(ap=srcg[:, gt:gt+1], axis=0))
                nc.gpsimd.indirect_dma_start(
                    out=d3[:, t, :], out_offset=None, in_=h3_sh[:],
                    in_offset=bass.IndirectOffsetOnAxis(ap=dstl[:, gt:gt+1], axis=0))
            e3 = small.tile([P, T, 1], F32, tag="e3")
            t3 = small.tile([P, T, 1], F32, tag="t3")
            nc.vector.tensor_tensor(out=e3, in0=g3, in1=a3s_b([P, T]),
                                    op=mybir.AluOpType.mult)
            nc.vector.tensor_tensor(out=t3, in0=d3, in1=a3d_b([P, T]),
                                    op=mybir.AluOpType.mult)
            nc.vector.tensor_tensor(out=e3, in0=e3, in1=t3, op=mybir.AluOpType.add)
            nc.vector.tensor_scalar_mul(t3, e3, NEG)
            nc.vector.tensor_tensor(out=e3, in0=e3, in1=t3, op=mybir.AluOpType.max)
            ex3 = small.tile([P, T, 1], F32, tag="ex3")
            nc.scalar.activation(out=ex3, in_=e3,
                                 func=mybir.ActivationFunctionType.Exp)
            m3 = small.tile([P, T, 2], F16, tag="m3")
            nc.vector.tensor_tensor(out=m3[:, :, 0:1], in0=ex3, in1=g3,
                                    op=mybir.AluOpType.mult)
            nc.vector.tensor_copy(out=m3[:, :, 1:2], in_=ex3)
            S = work.tile([P, T, P], F16, tag="S")
            db_b = bass.AP(tensor=dblk.tensor, offset=dblk[:, sl].offset,
                           ap=[dblk.ap[0], [dblk.ap[1][0], T], [0, P]])
            nc.vector.tensor_tensor(out=S, in0=db_b,
                                    in1=bcast_row(iot, [P, T, P]),
                                    op=mybir.AluOpType.is_equal)
            ps3f = pse.tile([P, HC + HEADS], F32, tag="agg")
            ps3 = ps3f[:, 0:2]
            for t in range(T):
                nc.tensor.matmul(ps3, S[:, t, :], m3[:, t, :],
                                 start=(t == 0), stop=(t == T - 1))
            den3 = small.tile([P, 1], F32, tag="den3")
            nc.vector.tensor_scalar_max(den3, ps3[:, 1:2], 1e-30)
            rec3 = small.tile([P, 1], F32, tag="rec3")
            nc.vector.reciprocal(out=rec3, in_=den3)
            nc.vector.tensor_tensor(out=outsb[:, b:b + 1], in0=ps3[:, 0:1],
                                    in1=rec3, op=mybir.AluOpType.mult)
        nc.sync.dma_start(out=out_p[:], in_=outsb)

    return nc


_CACHE = {}


def kernel(x, edge_index, W1, a_src1, a_dst1, b1, W2, a_src2, a_dst2, b2,
           W3, a_src3, a_dst3, b3):
    T, gslot, node_of_slot, srcg, dstl, dblk = _preprocess(np.asarray(edge_index))

    wa1 = _aug_weights(np.asarray(W1, np.float32), np.asarray(a_src1, np.float32),
                       np.asarray(a_dst1, np.float32), HEADS, HID)
    wa2 = _aug_weights(np.asarray(W2, np.float32), np.asarray(a_src2, np.float32),
                       np.asarray(a_dst2, np.float32), HEADS, HID)
    w3 = np.asarray(W3, np.float32).reshape(1, HC)
    c3 = np.array([[float(np.asarray(a_src3).reshape(-1)[0]),
                    float(np.asarray(a_dst3).reshape(-1)[0]),
                    float(np.asarray(b3).reshape(-1)[0]), 0.0]], np.float32)
    iota = np.arange(P, dtype=np.float16).reshape(1, P)
    b1r = np.asarray(b1, np.float32).reshape(1, HC)
    b2r = np.asarray(b2, np.float32).reshape(1, HC)

    x = np.asarray(x, np.float32)
    in_maps = []
    for c in range(NCORES):
        sl = slice(c * SLOTS, (c + 1) * SLOTS)
        nos = node_of_slot[sl]
        xs = np.zeros((SLOTS, IN_DIM), np.float32)
        valid = nos >= 0
        xs[valid] = x[nos[valid]]
        in_maps.append({
            "x_in": xs,
            "srcg_in": srcg[c], "dstl_in": dstl[c], "dblk_in": dblk[c],
            "wa1_in": wa1, "wa2_in": wa2, "w3_in": w3, "c3_in": c3,
            "b1_in": b1r, "b2_in": b2r, "iota_in": iota,
            "ident_in": np.eye(P, dtype=np.float32),
        })

    if T not in _CACHE:
        _CACHE[T] = _build(T)
    nc = _CACHE[T]
    res = run_bass_kernel_spmd(nc, in_maps, list(range(NCORES)))

    out = np.empty(N_NODES, np.float32)
    for c in range(NCORES):
        o = res.results[c]["out_p"]          # [P, NBLK]
        flat = o.T.reshape(-1)               # slot-major: b*P + p
        nos = node_of_slot[c * SLOTS:(c + 1) * SLOTS]
        valid = nos >= 0
        out[nos[valid]] = flat[valid]
    return out


# revision 13
# speedup vs baseline: 1.5329x; 1.5329x over previous
"""3-layer GAT (PyG GATConv semantics) on 8 Trainium2 NeuronCores.

Strategy (dst-sharded, gather-based):
- Nodes are assigned to 160 blocks of <=128 dst nodes, degree-balanced; 20 blocks per core.
- Per layer: each core computes its shard's dense projection h_aug = hprev @ [W | W@a_src | W@a_dst]
  (f32r matmuls), writes an fp16 row table [slots, 264], AllGathers the table.
- Edge phase per block: indirect-DMA gather of h_aug rows by edge src (fp16, 528B rows),
  indirect gather of alpha_dst rows by edge dst (f32, 16B rows), per-edge
  e = leaky(alpha_src + alpha_dst), ex = exp(e) (f32 math), messages m = ex * h (fp16),
  aggregation + softmax denominators via one PE matmul per 128-edge tile:
  lhsT = S (0/1 edge->dstslot matrix built by iota-compare), rhs = [m | ex] -> PSUM [128, 260].
- Softmax applied after aggregation: out = psum[:, :256] / denom (per head), + bias, ELU.
- Layer 3 (heads=1, C=1) same scheme with scalar tables.

The walrus in this toolchain accepts only ONE sync wait per instruction; BassOneWait
splits Tile-generated multi-waits into single-wait EventSemaphore ops at serialization.
"""
import numpy as np
from contextlib import ExitStack
import heapq

import orjson
import concourse.bass as bass
import concourse.tile as tile
from concourse import mybir
from concourse.bass_utils import run_bass_kernel_spmd

# problem constants (fixed by the harness's setup_inputs)
N_NODES = 20000
N_EDGES = 320000
IN_DIM = 128
HID = 64
HEADS = 4
HC = HEADS * HID          # 256
AUG = HC + 2 * HEADS      # 264 = h | alpha_src | alpha_dst
NEG = 0.2
NCORES = 8
P = 128
NBLK = 20                 # dst blocks per core
SLOTS = NBLK * P          # 2560 slots per core
TOT_SLOTS = SLOTS * NCORES

F32 = mybir.dt.float32
F32R = mybir.dt.float32r
F16 = mybir.dt.float16
I32 = mybir.dt.int32


def _split_multiwaits(bir: bytes) -> bytes:
    """Walrus here allows only 1 sync wait per instruction -> hoist extras onto
    same-engine EventSemaphore waits. Additionally, qPoolDynamic (SWDGE) DMA
    descriptors do not reliably honor embedded waits under this runtime ->
    hoist ALL their waits onto the issuing Pool engine."""
    j = orjson.loads(bir)
    ctr = 0
    for fn in j["functions"]:
        for blk in fn["blocks"]:
            out_l = []
            for ins in blk["instructions"]:
                si = ins.get("sync_info")
                ow = (si or {}).get("on_wait") or []
                hoist_all = ins.get("opcode") == "DMACopy" and ins.get("queue") == "qPoolDynamic"
                keep = 0 if hoist_all else 1
                if len(ow) > keep:
                    for w in ow[:len(ow) - keep]:
                        ctr += 1
                        out_l.append({
                            "engine": ins["engine"], "ins": [], "outs": [],
                            "name": f"mwsplit-{ctr}", "opcode": "EventSemaphore",
                            "sync_info": {"on_update": [], "on_wait": [w]},
                        })
                    si["on_wait"] = ow[len(ow) - keep:]
                out_l.append(ins)
            blk["instructions"] = out_l
    return orjson.dumps(j)


class BassOneWait(bass.Bass):
    def to_json_bytes(self):
        return _split_multiwaits(super().to_json_bytes())


# ---------------------------------------------------------------- host prep

def _preprocess(edge_index):
    """Assign nodes to degree-balanced blocks; build per-core edge tile arrays."""
    src = np.asarray(edge_index[0], dtype=np.int64)
    dst = np.asarray(edge_index[1], dtype=np.int64)
    loops = np.arange(N_NODES, dtype=np.int64)
    src = np.concatenate([src, loops])
    dst = np.concatenate([dst, loops])
    deg = np.bincount(dst, minlength=N_NODES).astype(np.int64)

    NB_TOT = NCORES * NBLK
    # greedy LPT: highest degree first onto least-loaded block with space
    order = np.argsort(-deg, kind="stable")
    blk_of = np.empty(N_NODES, np.int32)
    slot_of = np.empty(N_NODES, np.int32)
    heap = [(0, 0, b) for b in range(NB_TOT)]
    heapq.heapify(heap)
    cnt = np.zeros(NB_TOT, np.int32)
    load = np.zeros(NB_TOT, np.int64)
    for n in order:
        while True:
            l, _, b = heapq.heappop(heap)
            if cnt[b] < P:
                break
        blk_of[n] = b
        slot_of[n] = cnt[b]
        cnt[b] += 1
        load[b] += deg[n]
        if cnt[b] < P:
            heapq.heappush(heap, (load[b], cnt[b], b))

    T = int(np.ceil(load.max() / P))  # edge tiles per block (same for all)
    gslot = blk_of.astype(np.int64) * P + slot_of        # global table row of node
    node_of_slot = np.full(NB_TOT * P, -1, np.int64)
    node_of_slot[gslot] = np.arange(N_NODES)

    # bucket edges by dst block
    eb = blk_of[dst]
    order_e = np.argsort(eb, kind="stable")
    src_s = src[order_e]
    dst_s = dst[order_e]
    eb_s = eb[order_e]
    starts = np.searchsorted(eb_s, np.arange(NB_TOT + 1))

    NT = NBLK * T
    srcg = np.zeros((NCORES, P, NT), np.int32)       # global table row of edge src
    dstl = np.zeros((NCORES, P, NT), np.int32)       # core-local slot of edge dst
    dblk = np.full((NCORES, P, NT), -1.0, np.float16)  # block-local dst slot (-1 pad)
    for b in range(NB_TOT):
        c, lb = divmod(b, NBLK)
        e0, e1 = starts[b], starts[b + 1]
        k = e1 - e0
        col = np.zeros(T * P, np.int64)
        col[:k] = gslot[src_s[e0:e1]]
        srcg[c, :, lb * T:(lb + 1) * T] = col.reshape(T, P).T
        col_d = np.zeros(T * P, np.int64)
        col_d[:k] = lb * P + slot_of[dst_s[e0:e1]]
        dstl[c, :, lb * T:(lb + 1) * T] = col_d.reshape(T, P).T
        col_b = np.full(T * P, -1.0, np.float32)
        col_b[:k] = slot_of[dst_s[e0:e1]]
        dblk[c, :, lb * T:(lb + 1) * T] = col_b.reshape(T, P).T.astype(np.float16)

    return T, gslot, node_of_slot, srcg, dstl, dblk


def _aug_weights(W, a_src, a_dst, heads, hid):
    """[W | ws | wd] with ws[:,h] = W[:, h*hid:(h+1)*hid] @ a_src[h]."""
    cin = W.shape[0]
    ws = np.zeros((cin, heads), np.float32)
    wd = np.zeros((cin, heads), np.float32)
    for h in range(heads):
        blk = W[:, h * hid:(h + 1) * hid]
        ws[:, h] = blk @ a_src[h]
        wd[:, h] = blk @ a_dst[h]
    return np.concatenate([W, ws, wd], axis=1).astype(np.float32)


# ---------------------------------------------------------------- device kernel

def _build(T):
    NT = NBLK * T
    nc = BassOneWait()
    dp = nc.declare_dram_parameter
    x_in = dp("x_in", [SLOTS, IN_DIM], F32, isOutput=False)
    srcg_in = dp("srcg_in", [P, NT], I32, isOutput=False)
    dstl_in = dp("dstl_in", [P, NT], I32, isOutput=False)
    dblk_in = dp("dblk_in", [P, NT], F16, isOutput=False)
    wa1_in = dp("wa1_in", [IN_DIM, AUG], F32, isOutput=False)
    wa2_in = dp("wa2_in", [HC, AUG], F32, isOutput=False)
    w3_in = dp("w3_in", [1, HC], F32, isOutput=False)
    c3_in = dp("c3_in", [1, 4], F32, isOutput=False)   # a_src3, a_dst3, b3, 0
    b1_in = dp("b1_in", [1, HC], F32, isOutput=False)
    b2_in = dp("b2_in", [1, HC], F32, isOutput=False)
    iota_in = dp("iota_in", [1, P], F16, isOutput=False)
    ident_in = dp("ident_in", [P, P], F32, isOutput=False)
    out_p = dp("out_p", [P, NBLK], F32, isOutput=True)

    # internal DRAM
    tab_sh = [nc.dram_tensor(f"tab_sh{l}", [SLOTS, AUG], F16) for l in (1, 2)]
    tab_full = [nc.dram_tensor(f"tab_full{l}", [TOT_SLOTS, AUG], F16) for l in (1, 2)]
    adl_dram = [nc.dram_tensor(f"adl{l}", [SLOTS, HEADS], F32) for l in (1, 2)]
    h3_sh = nc.dram_tensor("h3_sh", [SLOTS, 1], F32)
    tab3 = nc.dram_tensor("tab3", [TOT_SLOTS, 1], F32)

    groups = [list(range(NCORES))]

    with tile.TileContext(nc) as tc, ExitStack() as ctx:
        consts = ctx.enter_context(tc.tile_pool(name="consts", bufs=1))
        meta = ctx.enter_context(tc.tile_pool(name="meta", bufs=1))
        state = ctx.enter_context(tc.tile_pool(name="state", bufs=1))
        work = ctx.enter_context(tc.tile_pool(name="work", bufs=2))
        gpool = ctx.enter_context(tc.tile_pool(name="gpool", bufs=3))
        small = ctx.enter_context(tc.tile_pool(name="small", bufs=4))
        psd = ctx.enter_context(tc.tile_pool(name="psd", bufs=1, space="PSUM"))
        pse = ctx.enter_context(tc.tile_pool(name="pse", bufs=2, space="PSUM"))
        pst = ctx.enter_context(tc.tile_pool(name="pst", bufs=2, space="PSUM"))
        psa = ctx.enter_context(tc.tile_pool(name="psa", bufs=2, space="PSUM"))

        # ---- constants / metadata loads
        ident = consts.tile([P, P], F32)
        nc.sync.dma_start(out=ident, in_=ident_in[:])
        ident16 = consts.tile([P, P], F16)
        nc.vector.tensor_copy(out=ident16, in_=ident)
        wa1 = consts.tile([P, AUG], F32)
        nc.sync.dma_start(out=wa1, in_=wa1_in[:])
        wa2 = consts.tile([P, 2, AUG], F32)
        nc.sync.dma_start(out=wa2, in_=wa2_in.rearrange("(j p) a -> p j a", p=P))
        def rep_load(name, src, n, dt):
            t = consts.tile([P, n], dt, tag=name)
            bc = bass.AP(tensor=src.tensor, offset=0, ap=[[0, P], [1, n]])
            nc.sync.dma_start(out=t, in_=bc)
            return t
        w3r = rep_load("w3r", w3_in[:], HC, F32)
        c3 = rep_load("c3", c3_in[:], 4, F32)
        b1r = rep_load("b1r", b1_in[:], HC, F32)
        b2r = rep_load("b2r", b2_in[:], HC, F32)
        iot = rep_load("iot", iota_in[:], P, F16)

        srcg = meta.tile([P, NT], I32)
        nc.sync.dma_start(out=srcg, in_=srcg_in[:])
        dstl = meta.tile([P, NT], I32)
        nc.sync.dma_start(out=dstl, in_=dstl_in[:])
        dblk = meta.tile([P, NT], F16)
        nc.sync.dma_start(out=dblk, in_=dblk_in[:])

        xin = state.tile([P, NBLK, IN_DIM], F32)
        nc.sync.dma_start(out=xin, in_=x_in.rearrange("(b p) d -> p b d", p=P))

        hprev = state.tile([P, NBLK, HC], F32)   # layer-1 output
        hprev2 = state.tile([P, NBLK, HC], F32)  # layer-2 output
        hT = state.tile([P, 2 * NBLK, P], F32)   # transposed dense input

        def bcast_row(t, shape):
            # t is [P, n] partition-replicated; broadcast middle dims (stride 0)
            ap = [list(t.ap[0])]
            for s in shape[1:-1]:
                ap.append([0, s])
            ap.append([t.ap[-1][0], shape[-1]])
            return bass.AP(tensor=t.tensor, offset=t.offset, ap=ap)

        adl_sbs = {}
        def dense_layer(lidx, cin_tiles):
            """h_aug per block -> tab_sh[lidx], adl_dram[lidx]."""
            adl_sb = state.tile([P, NBLK, HEADS], F16, tag=f"adl_sb{lidx}")
            adl_sbs[lidx] = adl_sb
            for b in range(NBLK):
                ps = psd.tile([P, AUG], F32, tag="dense")
                for j in range(cin_tiles):
                    lhsT = hT[:, cin_tiles * b + j, :]
                    rhs = wa1[:, :] if lidx == 0 else wa2[:, j, :]
                    nc.tensor.matmul(ps, lhsT, rhs,
                                     start=(j == 0), stop=(j == cin_tiles - 1))
                tabt = small.tile([P, AUG], F16, tag="tabt")
                nc.vector.tensor_copy(out=tabt, in_=ps)
                nc.sync.dma_start(
                    out=tab_sh[lidx].rearrange("(b p) a -> p b a", p=P)[:, b, :],
                    in_=tabt)
                nc.vector.tensor_copy(out=adl_sb[:, b, :], in_=ps[:, HC + HEADS:AUG])

        def transpose_into(src_view, dst_col):
            """PE-transpose [128,128] src_view into hT[:, dst_col, :]."""
            tp = pst.tile([P, P], F32, tag="tr")
            nc.tensor.transpose(out=tp, in_=src_view, identity=ident)
            nc.vector.tensor_copy(out=hT[:, dst_col, :], in_=tp)

        def edge_layer(lidx, hout, brow):
            """Gather + attention + aggregate for layer lidx (0 or 1)."""
            for b in range(NBLK):
                sl = slice(b * T, (b + 1) * T)
                hg = gpool.tile([P, T, AUG], F16, tag="hg")
                for t in range(T):
                    gt = b * T + t
                    nc.gpsimd.indirect_dma_start(
                        out=hg[:, t, :], out_offset=None, in_=tab_full[lidx][:],
                        in_offset=bass.IndirectOffsetOnAxis(ap=srcg[:, gt:gt+1], axis=0))
                # S first; then per-tile alpha_dst via PE: (S_t)^T @ adl_block
                S = work.tile([P, T, P], F16, tag="S")
                db_b = bass.AP(tensor=dblk.tensor, offset=dblk[:, sl].offset,
                               ap=[dblk.ap[0], [dblk.ap[1][0], T], [0, P]])
                nc.vector.tensor_tensor(out=S, in0=db_b,
                                        in1=bcast_row(iot, [P, T, P]),
                                        op=mybir.AluOpType.is_equal)
                adx = gpool.tile([P, T, HEADS], F32, tag="adx")
                adl_b = adl_sbs[lidx]
                for t in range(T):
                    stp = pst.tile([P, P], F16, tag="tr")
                    nc.tensor.transpose(out=stp, in_=S[:, t, :], identity=ident16)
                    stt = small.tile([P, P], F16, tag="stt")
                    nc.vector.tensor_copy(out=stt, in_=stp)
                    adp = psa.tile([P, HEADS], F32, tag="adp")
                    nc.tensor.matmul(adp, stt, adl_b[:, b, :], start=True, stop=True)
                    nc.vector.tensor_copy(out=adx[:, t, :], in_=adp)

                asum = small.tile([P, T, HEADS], F32, tag="asum")
                nc.vector.tensor_copy(out=asum, in_=hg[:, :, HC:HC + HEADS])
                nc.vector.tensor_tensor(out=asum, in0=asum, in1=adx,
                                        op=mybir.AluOpType.add)
                lk = small.tile([P, T, HEADS], F32, tag="lk")
                nc.vector.tensor_scalar_mul(lk, asum, NEG)
                nc.vector.tensor_tensor(out=lk, in0=lk, in1=asum,
                                        op=mybir.AluOpType.max)
                exf = small.tile([P, T, HEADS], F16, tag="exf")
                nc.scalar.activation(out=exf, in_=lk,
                                     func=mybir.ActivationFunctionType.Exp)

                m = work.tile([P, T, HC + HEADS], F16, tag="m")
                ex_b = bass.AP(tensor=exf.tensor, offset=exf.offset,
                               ap=[exf.ap[0], exf.ap[1], exf.ap[2], [0, HID]])
                nc.vector.tensor_tensor(
                    out=m[:, :, 0:HC].rearrange("p t (h c) -> p t h c", h=HEADS),
                    in0=hg[:, :, 0:HC].rearrange("p t (h c) -> p t h c", h=HEADS),
                    in1=ex_b, op=mybir.AluOpType.mult)
                nc.vector.tensor_copy(out=m[:, :, HC:HC + HEADS], in_=exf)

                ps = pse.tile([P, HC + HEADS], F32, tag="agg")
                for t in range(T):
                    nc.tensor.matmul(ps, S[:, t, :], m[:, t, :],
                                     start=(t == 0), stop=(t == T - 1))

                den = small.tile([P, HEADS], F32, tag="den")
                nc.vector.tensor_scalar_max(den, ps[:, HC:HC + HEADS], 1e-30)
                rec = small.tile([P, HEADS], F32, tag="rec")
                nc.vector.reciprocal(out=rec, in_=den)
                rec_b = bass.AP(tensor=rec.tensor, offset=rec.offset,
                                ap=[rec.ap[0], rec.ap[1], [0, HID]])
                hn = small.tile([P, HC], F32, tag="hn")
                nc.vector.tensor_tensor(
                    out=hn.rearrange("p (h c) -> p h c", h=HEADS),
                    in0=ps[:, 0:HC].rearrange("p (h c) -> p h c", h=HEADS),
                    in1=rec_b, op=mybir.AluOpType.mult)
                # bias + ELU
                nc.vector.tensor_tensor(out=hn, in0=hn, in1=brow,
                                        op=mybir.AluOpType.add)
                emin = small.tile([P, HC], F32, tag="emin")
                nc.vector.tensor_scalar_min(emin, hn, 0.0)
                eex = small.tile([P, HC], F32, tag="eex")
                nc.scalar.activation(out=eex, in_=emin,
                                     func=mybir.ActivationFunctionType.Exp)
                nc.vector.tensor_scalar_max(hn, hn, 0.0)
                nc.vector.tensor_tensor(out=hn, in0=hn, in1=eex,
                                        op=mybir.AluOpType.add)
                nc.vector.tensor_scalar_add(hout[:, b, :], hn, -1.0)

        # ================= layer 1
        for b in range(NBLK):
            transpose_into(xin[:, b, :], b)
        dense_layer(0, 1)
        nc.gpsimd.collective_compute(
            "AllGather", mybir.AluOpType.bypass, replica_groups=groups,
            ins=[tab_sh[0][:]], outs=[tab_full[0][:]])
        edge_layer(0, hprev, b1r)

        # ================= layer 2
        for b in range(NBLK):
            transpose_into(hprev[:, b, 0:P], 2 * b)
            transpose_into(hprev[:, b, P:HC], 2 * b + 1)
        dense_layer(1, 2)
        nc.gpsimd.collective_compute(
            "AllGather", mybir.AluOpType.bypass, replica_groups=groups,
            ins=[tab_sh[1][:]], outs=[tab_full[1][:]])
        edge_layer(1, hprev2, b2r)

        # ================= layer 3 dense: h3 = hprev2 @ W3 + b3
        h3sb = state.tile([P, NBLK, 1], F32)
        for b in range(NBLK):
            tmp = small.tile([P, HC], F32, tag="l3tmp")
            nc.vector.tensor_tensor(out=tmp, in0=hprev2[:, b, :],
                                    in1=w3r,
                                    op=mybir.AluOpType.mult)
            nc.vector.tensor_reduce(out=h3sb[:, b, :], in_=tmp,
                                    axis=mybir.AxisListType.X,
                                    op=mybir.AluOpType.add)
        h316 = state.tile([P, NBLK, 1], F16)
        nc.vector.tensor_copy(out=h316, in_=h3sb)
        b3_b = bass.AP(tensor=c3.tensor, offset=c3[:, 2:3].offset,
                       ap=[list(c3.ap[0]), [0, NBLK], [0, 1]])
        nc.vector.tensor_tensor(out=h3sb, in0=h3sb, in1=b3_b,
                                op=mybir.AluOpType.add)
        nc.sync.dma_start(out=h3_sh.rearrange("(b p) o -> p b o", p=P), in_=h3sb)
        nc.gpsimd.collective_compute(
            "AllGather", mybir.AluOpType.bypass, replica_groups=groups,
            ins=[h3_sh[:]], outs=[tab3[:]])

        # ================= layer 3 edge phase
        outsb = state.tile([P, NBLK], F32)
        a3s_b = lambda sh: bass.AP(tensor=c3.tensor, offset=c3[:, 0:1].offset,
                                   ap=[list(c3.ap[0]), [0, sh[1]], [0, 1]])
        a3d_b = lambda sh: bass.AP(tensor=c3.tensor, offset=c3[:, 1:2].offset,
                                   ap=[list(c3.ap[0]), [0, sh[1]], [0, 1]])
        for b in range(NBLK):
            sl = slice(b * T, (b + 1) * T)
            g3 = gpool.tile([P, T, 1], F32, tag="g3")
            d3 = gpool.tile([P, T, 1], F32, tag="d3")
            for t in range(T):
                gt = b * T + t
                nc.gpsimd.indirect_dma_start(
                    out=g3[:, t, :], out_offset=None, in_=tab3[:],
                    in_offset=bass.IndirectOffsetOnAxis(ap=srcg[:, gt:gt+1], axis=0))
            S = work.tile([P, T, P], F16, tag="S")
            db_b = bass.AP(tensor=dblk.tensor, offset=dblk[:, sl].offset,
                           ap=[dblk.ap[0], [dblk.ap[1][0], T], [0, P]])
            nc.vector.tensor_tensor(out=S, in0=db_b,
                                    in1=bcast_row(iot, [P, T, P]),
                                    op=mybir.AluOpType.is_equal)
            for t in range(T):
                stp = pst.tile([P, P], F16, tag="tr")
                nc.tensor.transpose(out=stp, in_=S[:, t, :], identity=ident16)
                stt = small.tile([P, P], F16, tag="stt")
                nc.vector.tensor_copy(out=stt, in_=stp)
                adp = psa.tile([P, HEADS], F32, tag="adp")
                nc.tensor.matmul(adp[:, 0:1], stt, h316[:, b, :], start=True, stop=True)
                nc.vector.tensor_copy(out=d3[:, t, :], in_=adp[:, 0:1])
            e3 = small.tile([P, T, 1], F32, tag="e3")
            t3 = small.tile([P, T, 1], F32, tag="t3")
            nc.vector.tensor_tensor(out=e3, in0=g3, in1=a3s_b([P, T]),
                                    op=mybir.AluOpType.mult)
            nc.vector.tensor_tensor(out=t3, in0=d3, in1=a3d_b([P, T]),
                                    op=mybir.AluOpType.mult)
            nc.vector.tensor_tensor(out=e3, in0=e3, in1=t3, op=mybir.AluOpType.add)
            nc.vector.tensor_scalar_mul(t3, e3, NEG)
            nc.vector.tensor_tensor(out=e3, in0=e3, in1=t3, op=mybir.AluOpType.max)
            ex3 = small.tile([P, T, 1], F32, tag="ex3")
            nc.scalar.activation(out=ex3, in_=e3,
                                 func=mybir.ActivationFunctionType.Exp)
            m3 = small.tile([P, T, 2], F16, tag="m3")
            nc.vector.tensor_tensor(out=m3[:, :, 0:1], in0=ex3, in1=g3,
                                    op=mybir.AluOpType.mult)
            nc.vector.tensor_copy(out=m3[:, :, 1:2], in_=ex3)
            ps3f = pse.tile([P, HC + HEADS], F32, tag="agg")
            ps3 = ps3f[:, 0:2]
            for t in range(T):
                nc.tensor.matmul(ps3, S[:, t, :], m3[:, t, :],
                                 start=(t == 0), stop=(t == T - 1))
            den3 = small.tile([P, 1], F32, tag="den3")
            nc.vector.tensor_scalar_max(den3, ps3[:, 1:2], 1e-30)
            rec3 = small.tile([P, 1], F32, tag="rec3")
            nc.vector.reciprocal(out=rec3, in_=den3)
            nc.vector.tensor_tensor(out=outsb[:, b:b + 1], in0=ps3[:, 0:1],
                                    in1=rec3, op=mybir.AluOpType.mult)
        nc.sync.dma_start(out=out_p[:], in_=outsb)

    return nc


_CACHE = {}


def kernel(x, edge_index, W1, a_src1, a_dst1, b1, W2, a_src2, a_dst2, b2,
           W3, a_src3, a_dst3, b3):
    T, gslot, node_of_slot, srcg, dstl, dblk = _preprocess(np.asarray(edge_index))

    wa1 = _aug_weights(np.asarray(W1, np.float32), np.asarray(a_src1, np.float32),
                       np.asarray(a_dst1, np.float32), HEADS, HID)
    wa2 = _aug_weights(np.asarray(W2, np.float32), np.asarray(a_src2, np.float32),
                       np.asarray(a_dst2, np.float32), HEADS, HID)
    w3 = np.asarray(W3, np.float32).reshape(1, HC)
    c3 = np.array([[float(np.asarray(a_src3).reshape(-1)[0]),
                    float(np.asarray(a_dst3).reshape(-1)[0]),
                    float(np.asarray(b3).reshape(-1)[0]), 0.0]], np.float32)
    iota = np.arange(P, dtype=np.float16).reshape(1, P)
    b1r = np.asarray(b1, np.float32).reshape(1, HC)
    b2r = np.asarray(b2, np.float32).reshape(1, HC)

    x = np.asarray(x, np.float32)
    in_maps = []
    for c in range(NCORES):
        sl = slice(c * SLOTS, (c + 1) * SLOTS)
        nos = node_of_slot[sl]
        xs = np.zeros((SLOTS, IN_DIM), np.float32)
        valid = nos >= 0
        xs[valid] = x[nos[valid]]
        in_maps.append({
            "x_in": xs,
            "srcg_in": srcg[c], "dstl_in": dstl[c], "dblk_in": dblk[c],
            "wa1_in": wa1, "wa2_in": wa2, "w3_in": w3, "c3_in": c3,
            "b1_in": b1r, "b2_in": b2r, "iota_in": iota,
            "ident_in": np.eye(P, dtype=np.float32),
        })

    if T not in _CACHE:
        _CACHE[T] = _build(T)
    nc = _CACHE[T]
    res = run_bass_kernel_spmd(nc, in_maps, list(range(NCORES)))

    out = np.empty(N_NODES, np.float32)
    for c in range(NCORES):
        o = res.results[c]["out_p"]          # [P, NBLK]
        flat = o.T.reshape(-1)               # slot-major: b*P + p
        nos = node_of_slot[c * SLOTS:(c + 1) * SLOTS]
        valid = nos >= 0
        out[nos[valid]] = flat[valid]
    return out
